# revision 2
# baseline (speedup 1.0000x reference)
"""Trainium2 Bass kernel: scatter rho[b, i, j] -> out[b, fock_idx[i], fock_idx[j]].

Sharding: batch dim B across the 8 NeuronCores (pure data parallel). fock_idx
is known on the host at call time, so the scatter addressing is baked into the
compiled program as static DMA/compute access patterns.

Per-core algorithm (out is [D, D], zero except out[idx[i], idx[j]] = rho[i, j];
the runtime hands the NEFF a zero-initialized ExternalOutput buffer, so only
rows/columns that receive data are written):
  - fock_idx decomposes into runs of consecutive indices (32 runs of 32 for
    the real problem). Each rho row expands into a [span]-wide SBUF row with
    the runs at their target offsets and zeros in the gaps; each 128-row tile
    then stores per row-run to the matching block of out rows, touching only
    columns [c0, c1).
  - R_all [128, 8*1024] holds all of rho persistently (no buffer recycling
    deps). One load DMA per tile: t0 on the sync HWDGE ring, t1 on scalar,
    the rest on gpsimd's SWDGE ring — HBM ramps immediately and the HWDGE
    rings stay clear for stores (no head-of-line blocking).
  - Expansion: column-run PAIRS as one 3-dim tensor_copy each ([rows, 2, L])
    — half the instruction count of per-run copies. Vector takes 11 of the
    16 pairs (it is ~1.4x faster per copy than GpSimd), GpSimd 5.
  - W bufs: 4 single-tile [128, span] buffers, memset once (W0/W1 up front on
    Vector, W2/W3 staggered on GpSimd behind early tiles), reused for tiles
    4-7: gap columns stay zero because copies only write the data columns;
    Tile's subtile tracking inserts the WAR deps on the store reads.
  - Stores: one DMA per row-run [L, span]. Keeping the row dim outermost
    matters: HWDGE sprays descriptors across all 16 SDMA engines keyed on
    the OUTER access-pattern dim (an outer dim of 2 pins a 516KB transfer to
    2 engines — measured 5x slowdown). Stores cycle across three issue paths
    (sync, scalar, gpsimd) so issue rate never caps the drain; 2 rings
    measurably bottleneck (~344 GB/s issue ceiling).
Measured ~46-50us/core on TRN2 (12.45 MB of HBM traffic/core at ~350 GB/s
plus ~9us fixed NEFF preamble/teardown), vs ~54us for the per-tile
Vector-serialized variant.
"""

import numpy as np

import concourse.bacc as bacc
import concourse.bass as bass
import concourse.mybir as mybir
from concourse import tile
from concourse.bass_utils import run_bass_kernel_spmd

N_CORES = 8
P = 128  # SBUF partitions
W_BUFS = 4
VCOPIES = 11  # column-run pairs copied on Vector; the rest on GpSimd


def _runs(dst, src):
    """Maximal runs where dst and src both advance by 1. Yields (d0, s0, len)."""
    out = []
    d0, s0, L = int(dst[0]), int(src[0]), 1
    for k in range(1, len(dst)):
        if int(dst[k]) == d0 + L and int(src[k]) == s0 + L:
            L += 1
        else:
            out.append((d0, s0, L))
            d0, s0, L = int(dst[k]), int(src[k]), 1
    out.append((d0, s0, L))
    return out


def _pair_runs(col_runs):
    """Group adjacent equal-length runs into stride-2 pairs.

    Returns a list of (dst0, src0, pair_dst_stride, pair_src_stride, n, L)
    where n is 1 or 2 repeats of an L-wide copy.
    """
    out = []
    k = 0
    while k < len(col_runs):
        d0, s0, L = col_runs[k]
        if k + 1 < len(col_runs) and col_runs[k + 1][2] == L:
            d1, s1, _ = col_runs[k + 1]
            out.append((d0, s0, d1 - d0, s1 - s0, 2, L))
            k += 2
        else:
            out.append((d0, s0, L, L, 1, L))
            k += 1
    return out


def _build(idx, D, n):
    """Build the per-core Bass program with idx baked in."""
    f32 = mybir.dt.float32

    order = np.argsort(idx, kind="stable")
    col_runs = _runs(idx[order], order)  # (dst_col, src_col, len)
    c0 = min(r[0] for r in col_runs)
    c1 = max(r[0] + r[2] for r in col_runs)
    span = c1 - c0
    col_pairs = _pair_runs(col_runs)

    n_tiles = (n + P - 1) // P

    nc = bacc.Bacc("TRN2", target_bir_lowering=False, debug=False,
                   num_devices=N_CORES)
    rho = nc.dram_tensor("rho", [n, n], f32, kind="ExternalInput")
    out = nc.dram_tensor("out", [D, D], f32, kind="ExternalOutput")

    with tile.TileContext(nc) as tc:
        with tc.tile_pool(name="sb", bufs=1) as sb:
            R = sb.tile([P, n_tiles * n], f32, name="R")
            ws = [sb.tile([P, span], f32, name=f"W{k}") for k in range(W_BUFS)]

            rings = [nc.sync, nc.scalar, nc.gpsimd]

            # Loads: every tile its own DMA; first two on the HWDGE rings
            # (earliest possible start), the rest on gpsimd's SWDGE ring.
            for t in range(n_tiles):
                rows = min(P, n - t * P)
                eng = nc.sync if t == 0 else (
                    nc.scalar if t == 1 else nc.gpsimd)
                eng.dma_start(R[:rows, t * n:t * n + n],
                              rho[t * P:t * P + rows, :])

            # W0/W1 memsets up front on Vector; W2/W3 staggered on GpSimd
            # behind early tiles (Vector is the busier engine).
            nc.vector.memset(ws[0][:], 0.0)
            if W_BUFS > 1:
                nc.vector.memset(ws[1][:], 0.0)
            next_ms = 2

            n_store = 0
            for t in range(n_tiles):
                rows = min(P, n - t * P)
                W = ws[t % W_BUFS]

                for k, (d0, s0, ds, ss, cnt, L) in enumerate(col_pairs):
                    eng = nc.vector if k < VCOPIES else nc.gpsimd
                    dst = bass.AP(W.tensor, W.offset + (d0 - c0),
                                  [[W.ap[0][0], rows], [ds, cnt], [1, L]])
                    src = bass.AP(R.tensor, R.offset + t * n + s0,
                                  [[R.ap[0][0], rows], [ss, cnt], [1, L]])
                    eng.tensor_copy(dst, src)

                while next_ms < W_BUFS and next_ms <= t + 2:
                    nc.gpsimd.memset(ws[next_ms][:], 0.0)
                    next_ms += 1

                for dr, sr, L in _runs(idx[t * P:t * P + rows], range(rows)):
                    ring = rings[n_store % len(rings)]
                    n_store += 1
                    ring.dma_start(out[dr:dr + L, c0:c1], W[sr:sr + L, :])
    nc.compile()
    return nc


def kernel(input_state, fock_idx, fock_dim):
    input_state = np.asarray(input_state)
    idx = np.asarray(fock_idx).astype(np.int64)
    D = int(fock_dim)
    B, n, _ = input_state.shape

    nc = _build(idx, D, n)

    out = np.empty((B, D, D), dtype=input_state.dtype)
    for start in range(0, B, N_CORES):
        stop = min(start + N_CORES, B)
        in_maps = [
            {"rho": np.ascontiguousarray(input_state[b], dtype=np.float32)}
            for b in range(start, stop)
        ]
        res = run_bass_kernel_spmd(nc, in_maps,
                                   core_ids=list(range(stop - start)))
        for k, b in enumerate(range(start, stop)):
            out[b] = res.results[k]["out"]
    return out
